# revision 1
# baseline (speedup 1.0000x reference)
"""Trainium2 Bass kernel for nn_BoxCrossCategoryLoss (8-core data-parallel), v2.

Math per row (36 relu terms):
    loss = sum_t relu(A_{a(t)} + B_{b(t)} - c_t)
with A/B/C p-values built from log-volumes via e = exp(v), l = ln(1-e),
and c in {C01,C11,C21,C31, L0,L1,L2}, L_k = ln(1-P_k) from AC products.

v2 layout strategy (everything bf16, slab-fused ops):
  per chunk of F columns ([128, F] tiles; 12 input cols packed [128,12F]):
    LL12 [12F]: Exp(v) for 10 cols -> ln(1-e) IN-PLACE; cols 10,11 = raw
                vAC1,vCA1 (DVE copy).  1 Ln op covers all 10 l's.
    ME   [4F] : [mAC, mCA, eAC0, eCA0];  Pq [3F] = [P1,P2,P0] (2 TT ops)
    cslab[9F] : [C01,L1,L2,C11,L0,L2b,C21,C31,L2c]  (2 TT + 2 Ln + 1 copy)
    AB   [14F]: A/B p-values, 4 paired TT ops (strided out APs)
    S    [14F]: 7 paired TT ops (broadcast in0)
    d    [15F + 21F]: 4 broadcast TT subs cover all 36 terms
    relu+accum in-place: 1 DVE tensor_scalar (4x) + 1-2 ACT Relu ops,
    split tuned by ACT_UNITS; per-partition partials -> stats (fp32).
  One manual ACT table load (natural_log_exp_and_others) -> no table thrash.
Host: pack per-core [128, 12, F] chunk slabs (bf16), pad rows so every
term's relu is exactly 0 on padding; sum stats in float64.
"""

import os
import sys

import numpy as np

for _p in ("/opt/trn_rl_repo", "/root/.axon_site/_ro/trn_rl_repo"):
    if os.path.isdir(_p) and _p not in sys.path:
        sys.path.insert(0, _p)

import ml_dtypes  # noqa: E402
import concourse.bacc as bacc  # noqa: E402
from concourse import mybir, tile  # noqa: E402
from concourse.bass_utils import run_bass_kernel_spmd  # noqa: E402
from concourse.hw_specs import get_activation_tables  # noqa: E402

BF16 = ml_dtypes.bfloat16
F32 = mybir.dt.float32
BF = mybir.dt.bfloat16
Alu = mybir.AluOpType
Act = mybir.ActivationFunctionType

N_CORES = 8
P = 128
CF = 768            # full-chunk width (columns of 128 rows)
ACT_SPLIT = 15      # F-units of d-slab relu handled by ACT; rest DVE
GP_ROWS_G1 = 2      # S-rows (of 5) of the G1 broadcast-sub done on GPSIMD
GP_ROWS_G2 = 1      # S-rows (of 5) of the G2 broadcast-sub done on GPSIMD
BUFS = {}           # per-pool buffer-count overrides, e.g. {"ss": 2}
GP_G34 = False      # route the G3/G4 subs (6F) to GPSIMD
GP_PQ = False       # route the AC product muls to GPSIMD
PE_G4 = True        # compute the 4 G4 terms on the TensorEngine via PSUM

# input column order inside each packed chunk [128, 12F]
COLS = ["AB0", "AB1", "BA0", "BA1", "BC0", "BC1", "CB0", "CB1",
        "AC1", "CA1", "AC0", "CA0"]
PAD_VAL = {"AB": -20.0, "BA": -20.0, "BC": -20.0, "CB": -20.0,
           "AC": -1e-3, "CA": -1e-3}
PAIR_NAMES = ["AB", "BA", "BC", "CB", "AC", "CA"]


def make_chunks(nf: int) -> list[int]:
    # "sf" (small chunk first) measured fastest: the tail chunk's dispatch
    # overhead hides in the pipeline ramp instead of serializing at the end.
    import os as _os
    mode = _os.environ.get("K2_CHUNKS", "sf")
    n = max(1, -(-nf // CF))
    if mode == "eq":
        base = -(-nf // n)
        base += base % 2
        chunks = [base] * (n - 1)
        last = nf - base * (n - 1)
        chunks.append(last)
    else:
        chunks = [CF] * (nf // CF)
        rem = nf - CF * len(chunks)
        if rem:
            if rem < 64 and chunks:      # avoid tiny tail chunks
                chunks[-1] += rem
            else:
                chunks.append(rem)
    if mode == "sf":
        chunks = sorted(chunks)
    assert sum(chunks) == nf and all(c % 2 == 0 for c in chunks)
    return chunks


def _act_set_id(nc) -> int:
    tabs = get_activation_tables(nc.m.arch)
    for i, name in enumerate(tabs):
        if "natural_log_exp" in name:
            return i
    return 0


def build_module(chunks: list[int]):
    nchunks = len(chunks)
    nc = bacc.Bacc("TRN2", target_bir_lowering=False, debug=False,
                   enable_asserts=False, num_devices=N_CORES)
    vin = [nc.dram_tensor(f"vin{k}", [P, 12 * Fk], BF, kind="ExternalInput")
           for k, Fk in enumerate(chunks)]
    if PE_G4:
        id_h = nc.dram_tensor("ident", [P, 2 * P], BF, kind="ExternalInput")
    NSLOT = 4
    out_h = nc.dram_tensor("stats", [P, NSLOT * nchunks], F32,
                           kind="ExternalOutput")

    # one combined Exp+Ln table load up front; the compile-time pass then
    # sees every activation satisfied and inserts no further loads.
    nc.scalar.add_instruction(mybir.InstLoadActFuncSet(
        name=nc.get_next_instruction_name(),
        act_func_set_id=_act_set_id(nc), ins=[], outs=[]))

    from contextlib import ExitStack
    with tile.TileContext(nc) as tc, ExitStack() as ctx:
        def pl(name, default):
            return ctx.enter_context(
                tc.tile_pool(name=name, bufs=BUFS.get(name, default)))
        vabp = pl("vab", 2)
        vacp = pl("vac", 2)
        e5p = pl("e5", 1)
        llp = pl("ll", 1)
        mep = pl("me", 1)
        pqp = pl("pq", 1)
        csp = pl("cs", 1)
        abp = pl("ab", 1)
        ssp = pl("ss", 1)
        d1p = pl("d1", 1)
        d2p = pl("d2", 1)
        stp = pl("st", 1)

        stats = stp.tile([P, NSLOT * nchunks], F32, tag="stats")

        if PE_G4:
            idp = pl("id", 1)
            psp = ctx.enter_context(
                tc.tile_pool(name="ps", bufs=BUFS.get("ps", 1), space="PSUM"))
            idt = idp.tile([P, 2 * P], BF, tag="idt")  # [I | -I]
            nc.sync.dma_start(idt[:], id_h.ap())

        for k, F in enumerate(chunks):
            vab = vabp.tile([P, 8 * F], BF, tag="vab")
            vac = vacp.tile([P, 4 * F], BF, tag="vac")
            nc.sync.dma_start(vab[:], vin[k].ap()[:, 0:8 * F])
            nc.sync.dma_start(vac[:], vin[k].ap()[:, 8 * F:12 * F])

            E5 = e5p.tile([P, 5 * F], F32, tag="E5")
            LL = llp.tile([P, 12 * F], BF, tag="LL")
            ME = mep.tile([P, 4 * F], F32, tag="ME")
            Pq = pqp.tile([P, 3 * F], F32, tag="Pq")
            cs = csp.tile([P, 9 * F], BF, tag="cs")
            AB = abp.tile([P, 14 * F], BF, tag="AB")
            S = ssp.tile([P, 14 * F], BF, tag="S")
            dG1 = d1p.tile([P, (17 if PE_G4 else 15) * F], BF, tag="dG1")
            dG2 = d2p.tile([P, (15 if PE_G4 else 21) * F], BF, tag="dG2")

            def bc(ap_col, outer, inner=None):
                """broadcast a [P, W] slice to [P, outer, W] (or [P,o,i,F])"""
                if inner is None:
                    return ap_col.rearrange("p (o f) -> p o f", o=1) \
                                 .broadcast_to([P, outer, ap_col.shape[-1]])
                W = ap_col.shape[-1] // inner
                return ap_col.rearrange("p (o c f) -> p o c f", o=1, c=inner) \
                             .broadcast_to([P, outer, inner, W])

            # ---- transcendentals ------------------------------------------
            # e must be fp32: bf16 rounds e=1-1e-6 to 1.0 and ln(1-e)=-inf.
            # Two 5-column substeps through one fp32 scratch bound SBUF use.
            nc.scalar.activation(E5[:], vab[:, 0:5 * F], Act.Exp)
            nc.scalar.activation(LL[:, 0:5 * F], E5[:], Act.Ln,
                                 bias=1.0, scale=-1.0)
            nc.scalar.activation(E5[:, 0:3 * F], vab[:, 5 * F:8 * F], Act.Exp)
            nc.scalar.activation(E5[:, 3 * F:5 * F], vac[:, 0:2 * F], Act.Exp)
            nc.scalar.activation(LL[:, 5 * F:10 * F], E5[:], Act.Ln,
                                 bias=1.0, scale=-1.0)
            nc.scalar.activation(ME[:, 2 * F:4 * F], vac[:, 2 * F:4 * F],
                                 Act.Exp)
            # raw vAC1, vCA1 next to the l's for paired C ops
            nc.vector.tensor_copy(LL[:, 10 * F:12 * F], vac[:, 0:2 * F])

            # ---- AC products & L values -----------------------------------
            # ME = [mAC, mCA, eAC0, eCA0];  m = 1 - e
            nc.vector.tensor_scalar(ME[:, 0:2 * F], ME[:, 2 * F:4 * F],
                                    -1.0, 1.0, Alu.mult, Alu.add)
            MEr = ME[:].rearrange("p (s f) -> p s f", s=4)
            Pqr = Pq[:].rearrange("p (s f) -> p s f", s=3)
            peng = nc.gpsimd if GP_PQ else nc.vector
            # [P1, P2] = [mAC, eAC0] * eCA0
            peng.tensor_tensor(Pqr[:, 0:2], MEr[:, 0:4:2],
                               bc(ME[:, 3 * F:4 * F], 2), Alu.mult)
            # P0 = eAC0 * mCA
            peng.tensor_tensor(Pqr[:, 2:3], MEr[:, 2:3], MEr[:, 1:2],
                               Alu.mult)
            # cslab = [C01, L1, L2, C11, L0, L2b, C21, C31, L2c]
            nc.scalar.activation(cs[:, 1 * F:3 * F], Pq[:, 0:2 * F], Act.Ln,
                                 bias=1.0, scale=-1.0)
            nc.scalar.activation(cs[:, 4 * F:5 * F], Pq[:, 2 * F:3 * F],
                                 Act.Ln, bias=1.0, scale=-1.0)

            LLr = LL[:].rearrange("p (s f) -> p s f", s=12)
            csr = cs[:].rearrange("p (s f) -> p s f", s=9)
            # {C01, C21} = vAC1 + [lCA1, vCA1]
            nc.vector.tensor_tensor(csr[:, 0:7:6], bc(LL[:, 10 * F:11 * F], 2),
                                    LLr[:, 9:12:2], Alu.add)
            # {C11, C31} = lAC1 + [vCA1, lCA1]
            nc.vector.tensor_tensor(csr[:, 3:8:4], bc(LL[:, 8 * F:9 * F], 2),
                                    LLr[:, 11:8:-2], Alu.add)
            # L2 copies to slots 5, 8
            nc.vector.tensor_copy(csr[:, 5:9:3], bc(cs[:, 2 * F:3 * F], 2))

            # ---- A/B p-values ---------------------------------------------
            # AB = [A00,A10,A20,A01,A11,A21,A31 | B00,B10,B20,B01,B11,B21,B31]
            Vg = vab[:].rearrange("p (g c f) -> p g c f", g=2, c=4)
            Lg = LL[:, 0:8 * F].rearrange("p (g c f) -> p g c f", g=2, c=4)
            ABg = AB[:].rearrange("p (g c f) -> p g c f", g=2, c=7)
            nc.vector.tensor_tensor(ABg[:, :, 0:4:3], Vg[:, :, 0:2],
                                    Lg[:, :, 2:4], Alu.add)   # A0c/B0c
            nc.vector.tensor_tensor(ABg[:, :, 1:5:3], Lg[:, :, 0:2],
                                    Vg[:, :, 2:4], Alu.add)   # A1c/B1c
            nc.vector.tensor_tensor(ABg[:, :, 2:6:3], Vg[:, :, 0:2],
                                    Vg[:, :, 2:4], Alu.add)   # A2c/B2c
            nc.vector.tensor_tensor(ABg[:, :, 6:7], Lg[:, :, 1:2],
                                    Lg[:, :, 3:4], Alu.add)   # A31/B31

            # ---- S sums ---------------------------------------------------
            # S = [S0,S1,S4,S8,S9 | S2,S3,S5,S10,S11 | S6,S12 | S7,S13]
            ABr = AB[:].rearrange("p (s f) -> p s f", s=14)
            Sr = S[:].rearrange("p (s f) -> p s f", s=14)
            A = {n: i for i, n in enumerate(
                ["A00", "A10", "A20", "A01", "A11", "A21", "A31",
                 "B00", "B10", "B20", "B01", "B11", "B21", "B31"])}

            def scol(i):
                return AB[:, i * F:(i + 1) * F]
            # quad-merged S ops via raw strided APs (7 ops -> 4)
            from concourse.ap import AP as _AP

            def sap(tile_ap, col0, dims):
                b = tile_ap
                full = [list(b.ap[0])] + [[s * F, n] for s, n in dims] \
                       + [[1, F]]
                return _AP(b.tensor, b.offset + col0 * F, full)

            # {S0,S1,S8,S9}: A(0,0,3,3) + B(10,12,7,9) -> S(0,1,3,4)
            nc.vector.tensor_tensor(
                sap(S[:], 0, [(3, 2), (1, 2)]),
                sap(AB[:], 0, [(3, 2), (0, 2)]),
                sap(AB[:], 10, [(-3, 2), (2, 2)]), Alu.add)
            # {S2,S3,S10,S11}: A(1,1,4,4) + B(11,12,8,9) -> S(5,6,8,9)
            nc.vector.tensor_tensor(
                sap(S[:], 5, [(3, 2), (1, 2)]),
                sap(AB[:], 1, [(3, 2), (0, 2)]),
                sap(AB[:], 11, [(-3, 2), (1, 2)]), Alu.add)
            # {S6,S7,S12,S13}: A(2,2,9,9) + B(12,13,5,6) -> S(10,12,11,13)
            nc.vector.tensor_tensor(
                sap(S[:], 10, [(1, 2), (2, 2)]),
                sap(AB[:], 2, [(7, 2), (0, 2)]),
                sap(AB[:], 12, [(-7, 2), (1, 2)]), Alu.add)
            # {S4,S5}: A20 + B(10,11) -> S(2,7)
            nc.vector.tensor_tensor(Sr[:, 2:8:5], bc(scol(A["A20"]), 2),
                                    ABr[:, 10:12], Alu.add)

            # ---- 36 subs in broadcast ops (DVE + GPSIMD split) ------------
            Sb = S[:].rearrange("p (s o f) -> p s o f", s=14, o=1)
            g1 = max(0, min(5, 5 - GP_ROWS_G1))   # DVE rows of G1
            g2 = max(0, min(5, 5 - GP_ROWS_G2))
            d1r = dG1[:, 0:15 * F].rearrange("p (s c f) -> p s c f",
                                             s=5, c=3)
            if g1 > 0:
                nc.vector.tensor_tensor(
                    d1r[:, 0:g1], Sb[:, 0:g1].broadcast_to([P, g1, 3, F]),
                    bc(cs[:, 0:3 * F], g1, inner=3), Alu.subtract)
            if g1 < 5:
                nc.gpsimd.tensor_tensor(
                    d1r[:, g1:5], Sb[:, g1:5].broadcast_to([P, 5 - g1, 3, F]),
                    bc(cs[:, 0:3 * F], 5 - g1, inner=3), Alu.subtract)
            d2r = dG2[:, 0:15 * F].rearrange("p (s c f) -> p s c f", s=5, c=3)
            if g2 > 0:
                nc.vector.tensor_tensor(
                    d2r[:, 0:g2], Sb[:, 5:5 + g2].broadcast_to([P, g2, 3, F]),
                    bc(cs[:, 3 * F:6 * F], g2, inner=3), Alu.subtract)
            if g2 < 5:
                nc.gpsimd.tensor_tensor(
                    d2r[:, g2:5],
                    Sb[:, 5 + g2:10].broadcast_to([P, 5 - g2, 3, F]),
                    bc(cs[:, 3 * F:6 * F], 5 - g2, inner=3), Alu.subtract)
            d3t = dG1[:, 15 * F:17 * F] if PE_G4 else dG2[:, 15 * F:17 * F]
            d3r = d3t.rearrange("p (s f) -> p s f", s=2)
            nc.vector.tensor_tensor(d3r, Sr[:, 10:12],
                                    bc(cs[:, 6 * F:7 * F], 2), Alu.subtract)
            if PE_G4:
                # G4 terms (S7,S13) x (C31,L2c) on the TensorEngine:
                # psum_slice = (-I)@c  then  += I@S ;  relu+accum on ACT.
                PS = psp.tile([P, 8 * 512], F32, tag="PS")
                Wh = F // 2
                g4_terms = [(12, 7), (12, 8), (13, 7), (13, 8)]
                for h in range(2):
                    for t, (scol, ccol) in enumerate(g4_terms):
                        sl = PS[:, (4 * h + t) * 512:(4 * h + t) * 512 + Wh]
                        nc.tensor.matmul(sl, idt[:, P:2 * P],
                                     cs[:, ccol * F + h * Wh:
                                        ccol * F + h * Wh + Wh],
                                     start=True, stop=False)
                for h in range(2):
                    for t, (scol, ccol) in enumerate(g4_terms):
                        sl = PS[:, (4 * h + t) * 512:(4 * h + t) * 512 + Wh]
                        nc.tensor.matmul(sl, idt[:, 0:P],
                                     S[:, scol * F + h * Wh:
                                       scol * F + h * Wh + Wh],
                                     start=False, stop=True)
            else:
                d4r = dG2[:, 17 * F:21 * F].rearrange("p (s c f) -> p s c f",
                                                      s=2, c=2)
                nc.vector.tensor_tensor(
                    d4r, Sb[:, 12:14].broadcast_to([P, 2, 2, F]),
                    bc(cs[:, 7 * F:9 * F], 2, inner=2), Alu.subtract)

            # ---- relu + accumulate ----------------------------------------
            d2w = 15 if PE_G4 else 21   # valid width of dG2 in F-units
            au = max(0, min(15 if PE_G4 else 36, ACT_SPLIT))
            au2 = min(au, d2w)         # ACT share of dG2
            au1 = au - au2             # ACT share of dG1 (prefix)
            s0 = stats[:, NSLOT * k:NSLOT * k + 1]
            s1 = stats[:, NSLOT * k + 1:NSLOT * k + 2]
            s2 = stats[:, NSLOT * k + 2:NSLOT * k + 3]
            s3 = stats[:, NSLOT * k + 3:NSLOT * k + 4]
            if au1 < 15:
                nc.vector.tensor_scalar(dG1[:, au1 * F:], dG1[:, au1 * F:],
                                        0.0, None, Alu.max, Alu.add,
                                        accum_out=s0)
            else:
                nc.vector.memset(s0, 0.0)
            # (in PE mode dG1 spans 17F incl. G3, handled by the op above)
            if au2 < d2w:
                nc.vector.tensor_scalar(dG2[:, au2 * F:d2w * F],
                                        dG2[:, au2 * F:d2w * F],
                                        0.0, None, Alu.max, Alu.add,
                                        accum_out=s1)
            else:
                nc.vector.memset(s1, 0.0)
            if au2 > 0:
                nc.scalar.activation(dG2[:, 0:au2 * F], dG2[:, 0:au2 * F],
                                     Act.Relu, accum_out=s2)
            else:
                nc.vector.memset(s2, 0.0)
            if PE_G4:
                PSr = PS[:].rearrange("p (s w) -> p s w", s=8)[:, :, 0:F // 2]
                nc.scalar.activation(PSr, PSr, Act.Relu, accum_out=s3)
            elif au1 > 0:
                nc.scalar.activation(dG1[:, 0:au1 * F], dG1[:, 0:au1 * F],
                                     Act.Relu, accum_out=s3)
            else:
                nc.vector.memset(s3, 0.0)

        nc.sync.dma_start(out_h.ap(), stats[:])

    nc.compile()
    return nc


_CACHE = {}


def _get_module(chunks):
    key = tuple(chunks)
    if key not in _CACHE:
        _CACHE[key] = build_module(list(chunks))
    return _CACHE[key]


LAST_RESULTS = None


def kernel(**inputs) -> np.ndarray:
    global LAST_RESULTS
    vols = {X: np.asarray(inputs["vol_" + X]) for X in PAIR_NAMES}
    n_rows = vols["AB"].shape[0]
    nf = -(-n_rows // (N_CORES * P))
    nf += nf % 2
    nf = max(nf, 128)
    chunks = make_chunks(nf)
    total_rows = N_CORES * P * nf

    # column arrays [N_CORES, P, nf] in bf16, padded
    colmap = {}
    for X in PAIR_NAMES:
        a = vols[X].astype(np.float32, copy=False)
        for c in (0, 1):
            col = np.full(total_rows, PAD_VAL[X], dtype=np.float32)
            col[:n_rows] = a[:, c]
            colmap[X + str(c)] = col.astype(BF16).reshape(N_CORES, P, nf)

    in_maps = [dict() for _ in range(N_CORES)]
    f0 = 0
    for k, F in enumerate(chunks):
        # [N_CORES, P, 12, F] slab per chunk
        slab = np.stack([colmap[c][:, :, f0:f0 + F] for c in COLS], axis=2)
        slab = np.ascontiguousarray(slab).reshape(N_CORES, P, 12 * F)
        for core in range(N_CORES):
            in_maps[core][f"vin{k}"] = slab[core]
        f0 += F

    if PE_G4:
        ident = np.zeros((P, 2 * P), dtype=BF16)
        ident[:, 0:P] = np.eye(P, dtype=BF16)
        ident[:, P:2 * P] = -np.eye(P, dtype=BF16)
        for core in range(N_CORES):
            in_maps[core]["ident"] = ident

    nc = _get_module(tuple(chunks))
    trace = bool(os.environ.get("BASS_TRACE"))
    if trace:
        try:
            from antenv import axon_hooks  # noqa: F401
        except ImportError:
            trace = False
    if not trace:
        os.environ["BASS_NEVER_TRACE"] = "1"
    res = run_bass_kernel_spmd(nc, in_maps, core_ids=list(range(N_CORES)),
                               trace=trace)
    LAST_RESULTS = res
    total = np.float64(0.0)
    for om in res.results:
        total += om["stats"].astype(np.float64).sum()
    return np.asarray(total, dtype=np.float32)


def _np_reference(ins):
    """float64 numpy reference for smoke testing."""
    def l1me(x):
        return np.log1p(-np.exp(x))
    p = {}
    for X, Y in [("AB", "BA"), ("BC", "CB"), ("AC", "CA")]:
        v1 = ins["vol_" + X].astype(np.float64)
        v2 = ins["vol_" + Y].astype(np.float64)
        l1, l2 = l1me(v1), l1me(v2)
        p[X] = [v1 + l2, l1 + v2, v1 + v2, l1 + l2]
    DM = {0: 0, 1: 0, 2: 0, 3: 0, 4: 1, 5: 1, 6: 1, 7: 1}
    LR = [(0, 4, 4), (0, 6, 4), (1, 5, 5), (1, 6, 5), (2, 4, 4), (2, 5, 5),
          (2, 6, 6), (2, 7, 7), (4, 0, 4), (4, 2, 4), (5, 1, 5), (5, 2, 5),
          (6, 2, 6), (7, 2, 7)]
    NR = [(0, 4, 1), (0, 4, 2), (0, 6, 1), (0, 6, 2), (1, 5, 0), (1, 5, 2),
          (1, 6, 0), (1, 6, 2), (2, 4, 1), (2, 4, 2), (2, 5, 0), (2, 5, 2),
          (4, 0, 1), (4, 0, 2), (4, 2, 1), (4, 2, 2), (5, 1, 0), (5, 1, 2),
          (5, 2, 0), (5, 2, 2), (2, 7, 2), (7, 2, 2)]
    loss = 0.0
    for xy, yz, xz in LR:
        t = (p["AB"][xy % 4][:, DM[xy]] + p["BC"][yz % 4][:, DM[yz]]
             - p["AC"][xz % 4][:, DM[xz]])
        loss += np.maximum(0.0, t).sum()
    for xy, yz, xz in NR:
        t = (p["AB"][xy % 4][:, DM[xy]] + p["BC"][yz % 4][:, DM[yz]]
             - l1me(p["AC"][xz % 4][:, DM[xz]]))
        loss += np.maximum(0.0, t).sum()
    return loss


if __name__ == "__main__":
    rng = np.random.default_rng(0)
    n = 300_000
    ins = {}
    for X in PAIR_NAMES:
        u = rng.uniform(1e-6, 1 - 1e-6, size=(n, 2)).astype(np.float32)
        ins["vol_" + X] = np.log(u)
    got = np.float64(kernel(**ins))
    exp = _np_reference(ins)
    print(f"kernel={got:.2f} ref={exp:.2f} rel={abs(got-exp)/abs(exp):.3e}")
    if LAST_RESULTS is not None and getattr(LAST_RESULTS, "exec_time_ns", None):
        print("exec_time_ns:", LAST_RESULTS.exec_time_ns)

